# revision 1
# baseline (speedup 1.0000x reference)
"""Trainium2 Bass kernel for ChronoRotationTransformation.

Computes, per batch row b (B=8192, D=2048):
    u   = (head_r + i*head_i) * (rel_r + i*rel_i)          # complex product
    ab  = sum_d u_r*tail_r - u_i*tail_i                    # == sum rot_r*t_r + rot_i*t_i
    aa  = sum_d u_r^2 + u_i^2                              # == |rot|^2
    bb  = sum_d tail_r^2 + tail_i^2
    out = ab / sqrt(aa*bb)

(The reference's rot = conj(head*rel); rot_r = u_r, rot_i = -u_i, so
ab = rot_r*t_r + rot_i*t_i = u_r*t_r - u_i*t_i and |rot|^2 = |u|^2.)

Sharding: pure data-parallel across 8 NeuronCores, 1024 rows each.
Per core: 8 row-tiles of [128, 2048]. DVE does the 4 cross products,
the two add/subs forming u, and two fused multiply+reduce (ab); ACT
does 4 square+accumulate reductions (aa, bb). Memory-bound target:
~48 MiB HBM reads per core.
"""

import numpy as np

B, D = 8192, 2048
NCORES = 8
BC = B // NCORES            # rows per core
P = 128                     # SBUF partitions
NT = BC // P                # row-tiles per core

IN_NAMES = [
    "head_real", "head_imag",
    "rel_real", "rel_imag",
    "tail_real", "tail_imag",
]

_CACHE = {}


def _emit(tc, ins, out_ap, mybir, repeats=1, cfg="v4"):
    import concourse.bass as bass  # noqa: F401

    nc = tc.nc
    f32 = mybir.dt.float32
    Alu = mybir.AluOpType
    Act = mybir.ActivationFunctionType

    # DRAM views: [NT, P, D] row-tiles; out as [P, NT] (row = t*128 + p).
    dv = {n: ins[n].rearrange("(t p) d -> t p d", p=P) for n in IN_NAMES}
    out_d = out_ap.rearrange("(t p) -> p t", p=P)

    with (
        tc.tile_pool(name="inp", bufs=2) as inp,
        tc.tile_pool(name="prod", bufs=1) as prod,
        tc.tile_pool(name="upool", bufs=2) as upool,
        tc.tile_pool(name="scr", bufs=1) as scr,
        tc.tile_pool(name="stats", bufs=1) as stats,
    ):
        ab1_s = stats.tile([P, NT], f32, tag="ab1_s")
        ab2_s = stats.tile([P, NT], f32, tag="ab2_s")
        aa1_s = stats.tile([P, NT], f32, tag="aa1_s")
        aa2_s = stats.tile([P, NT], f32, tag="aa2_s")
        bb1_s = stats.tile([P, NT], f32, tag="bb1_s")
        bb2_s = stats.tile([P, NT], f32, tag="bb2_s")

        for _rep in range(repeats):
          for t in range(NT):
            tiles = {}
            for n in IN_NAMES:
                # tail tiles are the last-released each tile (read by the
                # STT dots at the end) — give them one extra buffer so
                # their next DMA isn't gated on the ring.
                nb = 3 if (cfg == "v5" or n.startswith("tail")) else 2
                tl = inp.tile([P, D], f32, tag=n, bufs=nb)
                nc.sync.dma_start(out=tl[:], in_=dv[n][t])
                tiles[n] = tl
            hr, hi = tiles["head_real"], tiles["head_imag"]
            rr, ri = tiles["rel_real"], tiles["rel_imag"]
            tr, ti = tiles["tail_real"], tiles["tail_imag"]

            # All products on DVE: GPSIMD's fp32 tensor_tensor measured
            # ~4.5x slower than DVE here and coupling it into the tile
            # pipeline made the kernel slower, not faster.
            m3 = prod.tile([P, D], f32, tag="m3")
            nc.vector.tensor_mul(m3[:], hi[:], rr[:])
            m4 = prod.tile([P, D], f32, tag="m4")
            nc.vector.tensor_mul(m4[:], hr[:], ri[:])
            m1 = prod.tile([P, D], f32, tag="m1")
            nc.vector.tensor_mul(m1[:], hr[:], rr[:])
            m2 = prod.tile([P, D], f32, tag="m2")
            nc.vector.tensor_mul(m2[:], hi[:], ri[:])
            ub = 1 if cfg == "v5" else 2
            ur = upool.tile([P, D], f32, tag="ur", bufs=ub)
            nc.vector.tensor_sub(ur[:], m1[:], m2[:])
            ui = upool.tile([P, D], f32, tag="ui", bufs=ub)
            nc.vector.tensor_add(ui[:], m3[:], m4[:])

            # ab = sum(ur*tr) - sum(ui*ti): fused multiply+reduce via
            # scalar_tensor_tensor (out = (in0 op0 scalar) op1 in1,
            # accum_out = sum(out)). tensor_tensor_reduce (native TTR
            # opcode) crashes this terminal's NRT — do not use it.
            # Scratch outs alias the dead m1/m2 slots (same pool tag) —
            # WAR/WAW stay on-engine, zero extra SBUF.
            so1 = prod.tile([P, D], f32, tag="m1")
            nc.vector.scalar_tensor_tensor(
                out=so1[:], in0=ur[:], scalar=1.0, in1=tr[:],
                op0=Alu.mult, op1=Alu.mult, accum_out=ab1_s[:, t:t + 1],
            )
            so2 = prod.tile([P, D], f32, tag="m2")
            nc.vector.scalar_tensor_tensor(
                out=so2[:], in0=ui[:], scalar=-1.0, in1=ti[:],
                op0=Alu.mult, op1=Alu.mult, accum_out=ab2_s[:, t:t + 1],
            )

            # aa, bb: square+accumulate on ACT. bb first — tr/ti are
            # already resident before DVE finishes the products, so ACT
            # starts early and tr/ti stay hot for the STT dots.
            for src, dst in (
                (tr, bb1_s), (ti, bb2_s), (ur, aa1_s), (ui, aa2_s),
            ):
                ao = scr.tile([P, D], f32, tag="ao")
                nc.scalar.activation(
                    out=ao[:], in_=src[:], func=Act.Square,
                    accum_out=dst[:, t:t + 1],
                )

        # Final combine on [P, NT] (tiny).
        fin = {}
        def ftile(name):
            tl = stats.tile([P, NT], f32, tag=name)
            fin[name] = tl
            return tl

        ab = ftile("ab"); nc.vector.tensor_add(ab[:], ab1_s[:], ab2_s[:])
        aa = ftile("aa"); nc.vector.tensor_add(aa[:], aa1_s[:], aa2_s[:])
        bb = ftile("bb"); nc.vector.tensor_add(bb[:], bb1_s[:], bb2_s[:])
        pp = ftile("pp"); nc.vector.tensor_mul(pp[:], aa[:], bb[:])
        # sqrt on ACT is low precision (up to ~65536 ULP budget); refine
        # with two Newton iterations  r <- 0.5*(r + p/r)  using the
        # bit-exact DVE reciprocal.
        r = ftile("r0"); nc.scalar.activation(out=r[:], in_=pp[:], func=Act.Sqrt)
        for it in range(2):
            q = ftile(f"q{it}"); nc.vector.reciprocal(q[:], r[:])
            pq = ftile(f"pq{it}"); nc.vector.tensor_mul(pq[:], pp[:], q[:])
            s = ftile(f"s{it}"); nc.vector.tensor_add(s[:], r[:], pq[:])
            r = ftile(f"r{it + 1}"); nc.vector.tensor_scalar_mul(r[:], s[:], 0.5)
        inv = ftile("inv"); nc.vector.reciprocal(inv[:], r[:])
        score = ftile("score"); nc.vector.tensor_mul(score[:], ab[:], inv[:])
        nc.sync.dma_start(out=out_d, in_=score[:])


def _build(repeats=1, cfg="v4"):
    key = ("nc", repeats, cfg)
    if key in _CACHE:
        return _CACHE[key]
    import concourse.tile as tile
    from concourse import bacc, mybir

    # NOTE: num_devices is deliberately NOT set — it enables collective
    # global-comm setup that breaks plain SPMD input binding under the
    # axon/PJRT path (outputs come back as garbage).
    nc = bacc.Bacc(
        "TRN2",
        target_bir_lowering=False,
        debug=False,
    )
    ins = {
        n: nc.dram_tensor(n, [BC, D], mybir.dt.float32, kind="ExternalInput").ap()
        for n in IN_NAMES
    }
    out = nc.dram_tensor("out", [BC], mybir.dt.float32, kind="ExternalOutput").ap()
    with tile.TileContext(nc) as tc:
        _emit(tc, ins, out, mybir, repeats=repeats, cfg=cfg)
    nc.compile()
    _CACHE[key] = nc
    return nc


def run(inputs, trace=False, **kwargs):
    """Run on 8 cores; returns (full_output, BassKernelResults)."""
    from concourse.bass_utils import run_bass_kernel_spmd

    nc = _build()
    core_ids = list(range(NCORES))
    in_maps = []
    for c in range(NCORES):
        sl = slice(c * BC, (c + 1) * BC)
        in_maps.append(
            {n: np.ascontiguousarray(inputs[n][sl], dtype=np.float32)
             for n in IN_NAMES}
        )
    # The terminal occasionally reports the accelerator unrecoverable
    # (e.g. poisoned by an earlier crashed run); a fresh attempt after a
    # short wait triggers recovery.
    last_exc = None
    for attempt in range(4):
        try:
            res = run_bass_kernel_spmd(nc, in_maps, core_ids, trace=trace, **kwargs)
            break
        except Exception as e:  # noqa: BLE001
            last_exc = e
            if attempt == 3:
                raise
            import time as _time
            _time.sleep(15 * (attempt + 1))
    out = np.concatenate([res.results[c]["out"] for c in range(NCORES)])
    return out.astype(np.float32), res


def kernel(**inputs):
    out, _ = run(inputs)
    return out



# revision 19
# speedup vs baseline: 1.0749x; 1.0749x over previous
"""Trainium2 Bass kernel for ChronoRotationTransformation.

Computes, per batch row b (B=8192, D=2048):
    u   = (head_r + i*head_i) * (rel_r + i*rel_i)          # complex product
    ab  = sum_d u_r*tail_r - u_i*tail_i                    # == sum rot_r*t_r + rot_i*t_i
    aa  = sum_d u_r^2 + u_i^2                              # == |rot|^2
    bb  = sum_d tail_r^2 + tail_i^2
    out = ab / sqrt(aa*bb)

(The reference's rot = conj(head*rel); rot_r = u_r, rot_i = -u_i, so
ab = rot_r*t_r + rot_i*t_i = u_r*t_r - u_i*t_i and |rot|^2 = |u|^2.)

Sharding: pure data-parallel across 8 NeuronCores, 1024 rows each.
Per core: 8 row-tiles of [128, 2048]. DVE does the 4 cross products,
the two add/subs forming u, and two fused multiply+reduce (ab); ACT
does 4 square+accumulate reductions (aa, bb). Memory-bound target:
~48 MiB HBM reads per core.
"""

import numpy as np

B, D = 8192, 2048
NCORES = 8
BC = B // NCORES            # rows per core
P = 128                     # SBUF partitions
NT = BC // P                # row-tiles per core

IN_NAMES = [
    "head_real", "head_imag",
    "rel_real", "rel_imag",
    "tail_real", "tail_imag",
]

NTEN = len(IN_NAMES)

# Shipped configuration: all six input streams triple-buffered (deepest
# DMA lookahead that fits SBUF), ur/ui single-buffered. Measured ~1.1us
# faster than the v4 baseline (tail-only extra buffering); sits at the
# hardware's measured DMA floor (~347 GB/s/core effective).
DEFAULT_CFG = "v5"

# cfgs whose kernel takes one host-packed DRAM tensor instead of the six
# separate inputs. packc*: layout [NT, 6, P, D]; packr*: [NT, P, 6, D]
# (each partition line one contiguous 48 KB chunk).
PACKED_CFGS = ("packc", "packc_dmaonly", "packr", "packr_dmaonly")
ROWPACK_CFGS = ("packr", "packr_dmaonly")

_CACHE = {}


def host_pack(inputs, core, rowpack=False):
    """Pack core `core`'s slice of the 6 inputs into one array.
    rowpack=False: [t, n, p, d] (tile group = contiguous 6 MiB block).
    rowpack=True:  [t, p, n, d] (partition line = contiguous 48 KB)."""
    sl = slice(core * BC, (core + 1) * BC)
    arr = np.stack([np.asarray(inputs[n][sl], dtype=np.float32)
                    for n in IN_NAMES])            # [6, BC, D]
    arr = arr.reshape(NTEN, NT, P, D)
    arr = arr.transpose(1, 2, 0, 3) if rowpack else arr.transpose(1, 0, 2, 3)
    return np.ascontiguousarray(arr).reshape(NTEN * BC, D)


def host_prep(inputs, core, cfg):
    if cfg in PACKED_CFGS:
        return {"packed": host_pack(inputs, core, rowpack=cfg in ROWPACK_CFGS)}
    sl = slice(core * BC, (core + 1) * BC)
    return {n: np.ascontiguousarray(inputs[n][sl], dtype=np.float32)
            for n in IN_NAMES}


def _emit(tc, ins, out_ap, mybir, repeats=1, cfg="v4"):
    import concourse.bass as bass  # noqa: F401

    nc = tc.nc
    f32 = mybir.dt.float32
    Alu = mybir.AluOpType
    Act = mybir.ActivationFunctionType

    # DRAM views: [NT, P, D] row-tiles; out as [P, NT] (row = t*128 + p).
    if cfg not in PACKED_CFGS:
        dv = {n: ins[n].rearrange("(t p) d -> t p d", p=P) for n in IN_NAMES}
    out_d = out_ap.rearrange("(t p) -> p t", p=P)

    if cfg in PACKED_CFGS:
        # One DMA per row-tile group: all 6 tensors packed [t, n, p, d]
        # on the host, so each tile is a single contiguous 6 MiB read
        # split across all 16 SDMA engines. SBUF: [128, 6*2048] slabs.
        if cfg in ROWPACK_CFGS:
            pv = ins["packed"].rearrange("(t p n) d -> t p (n d)",
                                         n=NTEN, p=P)
        else:
            pv = ins["packed"].rearrange("(t n p) d -> t p n d",
                                         n=NTEN, p=P)
        dma_only = cfg.endswith("dmaonly")
        with (
            tc.tile_pool(name="inp", bufs=2) as inp,
            tc.tile_pool(name="prod", bufs=1) as prod,
            tc.tile_pool(name="upool", bufs=2) as upool,
            tc.tile_pool(name="scr", bufs=1) as scr,
            tc.tile_pool(name="stats", bufs=1) as stats,
        ):
            ab1_s = stats.tile([P, NT], f32, tag="ab1_s")
            ab2_s = stats.tile([P, NT], f32, tag="ab2_s")
            aa1_s = stats.tile([P, NT], f32, tag="aa1_s")
            aa2_s = stats.tile([P, NT], f32, tag="aa2_s")
            bb1_s = stats.tile([P, NT], f32, tag="bb1_s")
            bb2_s = stats.tile([P, NT], f32, tag="bb2_s")

            for _rep in range(repeats):
                for t in range(NT):
                    big = inp.tile([P, NTEN * D], f32, tag="big")
                    if cfg in ROWPACK_CFGS:
                        nc.sync.dma_start(out=big[:], in_=pv[t])
                    else:
                        big3 = big[:].rearrange("p (n d) -> p n d", n=NTEN)
                        nc.sync.dma_start(out=big3, in_=pv[t])
                    if dma_only:
                        continue
                    sl = {n: big[:, i * D:(i + 1) * D]
                          for i, n in enumerate(IN_NAMES)}
                    hr, hi = sl["head_real"], sl["head_imag"]
                    rr, ri = sl["rel_real"], sl["rel_imag"]
                    tr, ti = sl["tail_real"], sl["tail_imag"]

                    m3 = prod.tile([P, D], f32, tag="m3")
                    nc.vector.tensor_mul(m3[:], hi, rr)
                    m4 = prod.tile([P, D], f32, tag="m4")
                    nc.vector.tensor_mul(m4[:], hr, ri)
                    m1 = prod.tile([P, D], f32, tag="m1")
                    nc.vector.tensor_mul(m1[:], hr, rr)
                    m2 = prod.tile([P, D], f32, tag="m2")
                    nc.vector.tensor_mul(m2[:], hi, ri)
                    ur = upool.tile([P, D], f32, tag="ur")
                    nc.vector.tensor_sub(ur[:], m1[:], m2[:])
                    ui = upool.tile([P, D], f32, tag="ui")
                    nc.vector.tensor_add(ui[:], m3[:], m4[:])

                    so1 = prod.tile([P, D], f32, tag="m1")
                    nc.vector.scalar_tensor_tensor(
                        out=so1[:], in0=ur[:], scalar=1.0, in1=tr,
                        op0=Alu.mult, op1=Alu.mult,
                        accum_out=ab1_s[:, t:t + 1],
                    )
                    so2 = prod.tile([P, D], f32, tag="m2")
                    nc.vector.scalar_tensor_tensor(
                        out=so2[:], in0=ui[:], scalar=-1.0, in1=ti,
                        op0=Alu.mult, op1=Alu.mult,
                        accum_out=ab2_s[:, t:t + 1],
                    )

                    for src, dst in (
                        (tr, bb1_s), (ti, bb2_s),
                        (ur[:], aa1_s), (ui[:], aa2_s),
                    ):
                        ao = scr.tile([P, D], f32, tag="ao")
                        nc.scalar.activation(
                            out=ao[:], in_=src, func=Act.Square,
                            accum_out=dst[:, t:t + 1],
                        )

            if dma_only:
                score = stats.tile([P, NT], f32, tag="score")
                nc.vector.memset(score[:], 0.0)
                nc.sync.dma_start(out=out_d, in_=score[:])
                return

            fin = {}
            def ftile(name):
                tl = stats.tile([P, NT], f32, tag=name)
                fin[name] = tl
                return tl

            ab = ftile("ab"); nc.vector.tensor_add(ab[:], ab1_s[:], ab2_s[:])
            aa = ftile("aa"); nc.vector.tensor_add(aa[:], aa1_s[:], aa2_s[:])
            bb = ftile("bb"); nc.vector.tensor_add(bb[:], bb1_s[:], bb2_s[:])
            pp = ftile("pp"); nc.vector.tensor_mul(pp[:], aa[:], bb[:])
            r = ftile("r0"); nc.scalar.activation(out=r[:], in_=pp[:], func=Act.Sqrt)
            for it in range(2):
                q = ftile(f"q{it}"); nc.vector.reciprocal(q[:], r[:])
                pq = ftile(f"pq{it}"); nc.vector.tensor_mul(pq[:], pp[:], q[:])
                s = ftile(f"s{it}"); nc.vector.tensor_add(s[:], r[:], pq[:])
                r = ftile(f"r{it + 1}"); nc.vector.tensor_scalar_mul(r[:], s[:], 0.5)
            inv = ftile("inv"); nc.vector.reciprocal(inv[:], r[:])
            score = ftile("score"); nc.vector.tensor_mul(score[:], ab[:], inv[:])
            nc.sync.dma_start(out=out_d, in_=score[:])
        return

    if cfg in ("dmaonly", "dmaonly3", "dmaonly4", "dmaonly12"):
        # Diagnostic: input streaming only — the pure DMA/HBM floor.
        flat_nb = {"dmaonly3": 3, "dmaonly4": 4}.get(cfg)
        half = cfg == "dmaonly12"
        with (
            tc.tile_pool(name="inp", bufs=2) as inp,
            tc.tile_pool(name="stats", bufs=1) as stats,
        ):
            for _rep in range(repeats):
                for t in range(NT):
                    for n in IN_NAMES:
                        nb = flat_nb or (3 if n.startswith("tail") else 2)
                        tl = inp.tile([P, D], f32, tag=n, bufs=nb)
                        if half:
                            h = D // 2
                            nc.sync.dma_start(out=tl[:, :h],
                                              in_=dv[n][t][:, :h])
                            nc.sync.dma_start(out=tl[:, h:],
                                              in_=dv[n][t][:, h:])
                        else:
                            nc.sync.dma_start(out=tl[:], in_=dv[n][t])
            score = stats.tile([P, NT], f32, tag="score")
            nc.vector.memset(score[:], 0.0)
            nc.sync.dma_start(out=out_d, in_=score[:])
        return

    if cfg == "computeonly":
        # Diagnostic: same compute chain, no input DMAs (static tiles).
        with (
            tc.tile_pool(name="inp", bufs=1) as inp,
            tc.tile_pool(name="prod", bufs=1) as prod,
            tc.tile_pool(name="upool", bufs=2) as upool,
            tc.tile_pool(name="scr", bufs=1) as scr,
            tc.tile_pool(name="stats", bufs=1) as stats,
        ):
            tiles = {}
            for n in IN_NAMES:
                tl = inp.tile([P, D], f32, tag=n)
                nc.vector.memset(tl[:], 0.25)
                tiles[n] = tl
            hr, hi = tiles["head_real"], tiles["head_imag"]
            rr, ri = tiles["rel_real"], tiles["rel_imag"]
            tr, ti = tiles["tail_real"], tiles["tail_imag"]
            ab1_s = stats.tile([P, NT], f32, tag="ab1_s")
            ab2_s = stats.tile([P, NT], f32, tag="ab2_s")
            aa1_s = stats.tile([P, NT], f32, tag="aa1_s")
            aa2_s = stats.tile([P, NT], f32, tag="aa2_s")
            bb1_s = stats.tile([P, NT], f32, tag="bb1_s")
            bb2_s = stats.tile([P, NT], f32, tag="bb2_s")
            for _rep in range(repeats):
                for t in range(NT):
                    m3 = prod.tile([P, D], f32, tag="m3")
                    nc.vector.tensor_mul(m3[:], hi[:], rr[:])
                    m4 = prod.tile([P, D], f32, tag="m4")
                    nc.vector.tensor_mul(m4[:], hr[:], ri[:])
                    m1 = prod.tile([P, D], f32, tag="m1")
                    nc.vector.tensor_mul(m1[:], hr[:], rr[:])
                    m2 = prod.tile([P, D], f32, tag="m2")
                    nc.vector.tensor_mul(m2[:], hi[:], ri[:])
                    ur = upool.tile([P, D], f32, tag="ur")
                    nc.vector.tensor_sub(ur[:], m1[:], m2[:])
                    ui = upool.tile([P, D], f32, tag="ui")
                    nc.vector.tensor_add(ui[:], m3[:], m4[:])
                    so1 = prod.tile([P, D], f32, tag="m1")
                    nc.vector.scalar_tensor_tensor(
                        out=so1[:], in0=ur[:], scalar=1.0, in1=tr[:],
                        op0=Alu.mult, op1=Alu.mult,
                        accum_out=ab1_s[:, t:t + 1],
                    )
                    so2 = prod.tile([P, D], f32, tag="m2")
                    nc.vector.scalar_tensor_tensor(
                        out=so2[:], in0=ui[:], scalar=-1.0, in1=ti[:],
                        op0=Alu.mult, op1=Alu.mult,
                        accum_out=ab2_s[:, t:t + 1],
                    )
                    for src, dst in (
                        (tr, bb1_s), (ti, bb2_s), (ur, aa1_s), (ui, aa2_s),
                    ):
                        ao = scr.tile([P, D], f32, tag="ao")
                        nc.scalar.activation(
                            out=ao[:], in_=src[:], func=Act.Square,
                            accum_out=dst[:, t:t + 1],
                        )
            score = stats.tile([P, NT], f32, tag="score")
            nc.vector.memset(score[:], 0.0)
            nc.sync.dma_start(out=out_d, in_=score[:])
        return

    with (
        tc.tile_pool(name="inp", bufs=2) as inp,
        tc.tile_pool(name="prod", bufs=1) as prod,
        tc.tile_pool(name="upool", bufs=2) as upool,
        tc.tile_pool(name="scr", bufs=1) as scr,
        tc.tile_pool(name="stats", bufs=1) as stats,
    ):
        ab1_s = stats.tile([P, NT], f32, tag="ab1_s")
        ab2_s = stats.tile([P, NT], f32, tag="ab2_s")
        aa1_s = stats.tile([P, NT], f32, tag="aa1_s")
        aa2_s = stats.tile([P, NT], f32, tag="aa2_s")
        bb1_s = stats.tile([P, NT], f32, tag="bb1_s")
        bb2_s = stats.tile([P, NT], f32, tag="bb2_s")

        for _rep in range(repeats):
          for t in range(NT):
            tiles = {}
            for i, n in enumerate(IN_NAMES):
                # tail tiles are the last-released each tile (read by the
                # STT dots at the end) — give them one extra buffer so
                # their next DMA isn't gated on the ring.
                if cfg == "v5":
                    nb = 3
                elif cfg == "v7":
                    nb = 4 if n.startswith("tail") else 2
                else:
                    nb = 3 if n.startswith("tail") else 2
                tl = inp.tile([P, D], f32, tag=n, bufs=nb)
                # v8: alternate the two HWDGE rings (SP and ACT) so
                # descriptor generation / completion handling overlap.
                eng = nc.scalar if (cfg == "v8" and i % 2) else nc.sync
                eng.dma_start(out=tl[:], in_=dv[n][t])
                tiles[n] = tl
            hr, hi = tiles["head_real"], tiles["head_imag"]
            rr, ri = tiles["rel_real"], tiles["rel_imag"]
            tr, ti = tiles["tail_real"], tiles["tail_imag"]

            # All products on DVE: GPSIMD's fp32 tensor_tensor measured
            # ~4.5x slower than DVE here and coupling it into the tile
            # pipeline made the kernel slower, not faster.
            m3 = prod.tile([P, D], f32, tag="m3")
            nc.vector.tensor_mul(m3[:], hi[:], rr[:])
            m4 = prod.tile([P, D], f32, tag="m4")
            nc.vector.tensor_mul(m4[:], hr[:], ri[:])
            m1 = prod.tile([P, D], f32, tag="m1")
            nc.vector.tensor_mul(m1[:], hr[:], rr[:])
            m2 = prod.tile([P, D], f32, tag="m2")
            nc.vector.tensor_mul(m2[:], hi[:], ri[:])
            ub = 1 if cfg == "v5" else 2
            ur = upool.tile([P, D], f32, tag="ur", bufs=ub)
            nc.vector.tensor_sub(ur[:], m1[:], m2[:])
            ui = upool.tile([P, D], f32, tag="ui", bufs=ub)
            nc.vector.tensor_add(ui[:], m3[:], m4[:])

            # ab = sum(ur*tr) - sum(ui*ti): fused multiply+reduce via
            # scalar_tensor_tensor (out = (in0 op0 scalar) op1 in1,
            # accum_out = sum(out)). tensor_tensor_reduce (native TTR
            # opcode) crashes this terminal's NRT — do not use it.
            # Scratch outs alias the dead m1/m2 slots (same pool tag) —
            # WAR/WAW stay on-engine, zero extra SBUF.
            so1 = prod.tile([P, D], f32, tag="m1")
            nc.vector.scalar_tensor_tensor(
                out=so1[:], in0=ur[:], scalar=1.0, in1=tr[:],
                op0=Alu.mult, op1=Alu.mult, accum_out=ab1_s[:, t:t + 1],
            )
            so2 = prod.tile([P, D], f32, tag="m2")
            nc.vector.scalar_tensor_tensor(
                out=so2[:], in0=ui[:], scalar=-1.0, in1=ti[:],
                op0=Alu.mult, op1=Alu.mult, accum_out=ab2_s[:, t:t + 1],
            )

            # aa, bb: square+accumulate on ACT. bb first — tr/ti are
            # already resident before DVE finishes the products, so ACT
            # starts early and tr/ti stay hot for the STT dots.
            for src, dst in (
                (tr, bb1_s), (ti, bb2_s), (ur, aa1_s), (ui, aa2_s),
            ):
                ao = scr.tile([P, D], f32, tag="ao")
                nc.scalar.activation(
                    out=ao[:], in_=src[:], func=Act.Square,
                    accum_out=dst[:, t:t + 1],
                )

        # Final combine on [P, NT] (tiny).
        fin = {}
        def ftile(name):
            tl = stats.tile([P, NT], f32, tag=name)
            fin[name] = tl
            return tl

        ab = ftile("ab"); nc.vector.tensor_add(ab[:], ab1_s[:], ab2_s[:])
        aa = ftile("aa"); nc.vector.tensor_add(aa[:], aa1_s[:], aa2_s[:])
        bb = ftile("bb"); nc.vector.tensor_add(bb[:], bb1_s[:], bb2_s[:])
        pp = ftile("pp"); nc.vector.tensor_mul(pp[:], aa[:], bb[:])
        # sqrt on ACT is low precision (up to ~65536 ULP budget); refine
        # with two Newton iterations  r <- 0.5*(r + p/r)  using the
        # bit-exact DVE reciprocal.
        r = ftile("r0"); nc.scalar.activation(out=r[:], in_=pp[:], func=Act.Sqrt)
        for it in range(2):
            q = ftile(f"q{it}"); nc.vector.reciprocal(q[:], r[:])
            pq = ftile(f"pq{it}"); nc.vector.tensor_mul(pq[:], pp[:], q[:])
            s = ftile(f"s{it}"); nc.vector.tensor_add(s[:], r[:], pq[:])
            r = ftile(f"r{it + 1}"); nc.vector.tensor_scalar_mul(r[:], s[:], 0.5)
        inv = ftile("inv"); nc.vector.reciprocal(inv[:], r[:])
        score = ftile("score"); nc.vector.tensor_mul(score[:], ab[:], inv[:])
        nc.sync.dma_start(out=out_d, in_=score[:])


def _build(repeats=1, cfg="v4"):
    key = ("nc", repeats, cfg)
    if key in _CACHE:
        return _CACHE[key]
    import concourse.tile as tile
    from concourse import bacc, mybir

    # NOTE: num_devices is deliberately NOT set — it enables collective
    # global-comm setup that breaks plain SPMD input binding under the
    # axon/PJRT path (outputs come back as garbage).
    nc = bacc.Bacc(
        "TRN2",
        target_bir_lowering=False,
        debug=False,
    )
    if cfg in PACKED_CFGS:
        ins = {
            "packed": nc.dram_tensor(
                "packed", [NTEN * BC, D], mybir.dt.float32,
                kind="ExternalInput",
            ).ap()
        }
    else:
        ins = {
            n: nc.dram_tensor(n, [BC, D], mybir.dt.float32, kind="ExternalInput").ap()
            for n in IN_NAMES
        }
    out = nc.dram_tensor("out", [BC], mybir.dt.float32, kind="ExternalOutput").ap()
    with tile.TileContext(nc) as tc:
        _emit(tc, ins, out, mybir, repeats=repeats, cfg=cfg)
    nc.compile()
    _CACHE[key] = nc
    return nc


def run(inputs, trace=False, cfg=DEFAULT_CFG, **kwargs):
    """Run on 8 cores; returns (full_output, BassKernelResults)."""
    from concourse.bass_utils import run_bass_kernel_spmd

    nc = _build(cfg=cfg)
    core_ids = list(range(NCORES))
    in_maps = [host_prep(inputs, c, cfg) for c in range(NCORES)]
    # The terminal occasionally reports the accelerator unrecoverable
    # (e.g. poisoned by an earlier crashed run); a fresh attempt after a
    # short wait triggers recovery.
    last_exc = None
    for attempt in range(4):
        try:
            res = run_bass_kernel_spmd(nc, in_maps, core_ids, trace=trace, **kwargs)
            break
        except Exception as e:  # noqa: BLE001
            last_exc = e
            if attempt == 3:
                raise
            import time as _time
            _time.sleep(15 * (attempt + 1))
    out = np.concatenate([res.results[c]["out"] for c in range(NCORES)])
    return out.astype(np.float32), res


def kernel(**inputs):
    out, _ = run(inputs)
    return out



# revision 21
# speedup vs baseline: 1.0820x; 1.0066x over previous
"""Trainium2 Bass kernel for ChronoRotationTransformation.

Computes, per batch row b (B=8192, D=2048):
    u   = (head_r + i*head_i) * (rel_r + i*rel_i)          # complex product
    ab  = sum_d u_r*tail_r - u_i*tail_i                    # == sum rot_r*t_r + rot_i*t_i
    aa  = sum_d u_r^2 + u_i^2                              # == |rot|^2
    bb  = sum_d tail_r^2 + tail_i^2
    out = ab / sqrt(aa*bb)

(The reference's rot = conj(head*rel); rot_r = u_r, rot_i = -u_i, so
ab = rot_r*t_r + rot_i*t_i = u_r*t_r - u_i*t_i and |rot|^2 = |u|^2.)

Sharding: pure data-parallel across 8 NeuronCores, 1024 rows each.
Per core: 8 row-tiles of [128, 2048]. DVE does the 4 cross products,
the two add/subs forming u, and two fused multiply+reduce (ab); ACT
does 4 square+accumulate reductions (aa, bb). Memory-bound target:
~48 MiB HBM reads per core.
"""

import numpy as np

B, D = 8192, 2048
NCORES = 8
BC = B // NCORES            # rows per core
P = 128                     # SBUF partitions
NT = BC // P                # row-tiles per core

IN_NAMES = [
    "head_real", "head_imag",
    "rel_real", "rel_imag",
    "tail_real", "tail_imag",
]

NTEN = len(IN_NAMES)

# Shipped configuration: all six input streams triple-buffered (deepest
# DMA lookahead that fits SBUF), ur/ui single-buffered. Measured ~1.1us
# faster than the v4 baseline (tail-only extra buffering); sits at the
# hardware's measured DMA floor (~347 GB/s/core effective).
DEFAULT_CFG = "v5"

# cfgs whose kernel takes one host-packed DRAM tensor instead of the six
# separate inputs. packc*: layout [NT, 6, P, D]; packr*: [NT, P, 6, D]
# (each partition line one contiguous 48 KB chunk).
PACKED_CFGS = ("packc", "packc_dmaonly", "packr", "packr_dmaonly")
ROWPACK_CFGS = ("packr", "packr_dmaonly")

_CACHE = {}


def host_pack(inputs, core, rowpack=False):
    """Pack core `core`'s slice of the 6 inputs into one array.
    rowpack=False: [t, n, p, d] (tile group = contiguous 6 MiB block).
    rowpack=True:  [t, p, n, d] (partition line = contiguous 48 KB)."""
    sl = slice(core * BC, (core + 1) * BC)
    arr = np.stack([np.asarray(inputs[n][sl], dtype=np.float32)
                    for n in IN_NAMES])            # [6, BC, D]
    arr = arr.reshape(NTEN, NT, P, D)
    arr = arr.transpose(1, 2, 0, 3) if rowpack else arr.transpose(1, 0, 2, 3)
    return np.ascontiguousarray(arr).reshape(NTEN * BC, D)


def host_prep(inputs, core, cfg):
    if cfg in PACKED_CFGS:
        return {"packed": host_pack(inputs, core, rowpack=cfg in ROWPACK_CFGS)}
    sl = slice(core * BC, (core + 1) * BC)
    return {n: np.ascontiguousarray(inputs[n][sl], dtype=np.float32)
            for n in IN_NAMES}


def _emit(tc, ins, out_ap, mybir, repeats=1, cfg="v4"):
    import concourse.bass as bass  # noqa: F401

    nc = tc.nc
    f32 = mybir.dt.float32
    Alu = mybir.AluOpType
    Act = mybir.ActivationFunctionType

    # DRAM views: [NT, P, D] row-tiles; out as [P, NT] (row = t*128 + p).
    if cfg not in PACKED_CFGS:
        dv = {n: ins[n].rearrange("(t p) d -> t p d", p=P) for n in IN_NAMES}
    out_d = out_ap.rearrange("(t p) -> p t", p=P)

    if cfg in PACKED_CFGS:
        # One DMA per row-tile group: all 6 tensors packed [t, n, p, d]
        # on the host, so each tile is a single contiguous 6 MiB read
        # split across all 16 SDMA engines. SBUF: [128, 6*2048] slabs.
        if cfg in ROWPACK_CFGS:
            pv = ins["packed"].rearrange("(t p n) d -> t p (n d)",
                                         n=NTEN, p=P)
        else:
            pv = ins["packed"].rearrange("(t n p) d -> t p n d",
                                         n=NTEN, p=P)
        dma_only = cfg.endswith("dmaonly")
        with (
            tc.tile_pool(name="inp", bufs=2) as inp,
            tc.tile_pool(name="prod", bufs=1) as prod,
            tc.tile_pool(name="upool", bufs=2) as upool,
            tc.tile_pool(name="scr", bufs=1) as scr,
            tc.tile_pool(name="stats", bufs=1) as stats,
        ):
            ab1_s = stats.tile([P, NT], f32, tag="ab1_s")
            ab2_s = stats.tile([P, NT], f32, tag="ab2_s")
            aa1_s = stats.tile([P, NT], f32, tag="aa1_s")
            aa2_s = stats.tile([P, NT], f32, tag="aa2_s")
            bb1_s = stats.tile([P, NT], f32, tag="bb1_s")
            bb2_s = stats.tile([P, NT], f32, tag="bb2_s")

            for _rep in range(repeats):
                for t in range(NT):
                    big = inp.tile([P, NTEN * D], f32, tag="big")
                    if cfg in ROWPACK_CFGS:
                        nc.sync.dma_start(out=big[:], in_=pv[t])
                    else:
                        big3 = big[:].rearrange("p (n d) -> p n d", n=NTEN)
                        nc.sync.dma_start(out=big3, in_=pv[t])
                    if dma_only:
                        continue
                    sl = {n: big[:, i * D:(i + 1) * D]
                          for i, n in enumerate(IN_NAMES)}
                    hr, hi = sl["head_real"], sl["head_imag"]
                    rr, ri = sl["rel_real"], sl["rel_imag"]
                    tr, ti = sl["tail_real"], sl["tail_imag"]

                    m3 = prod.tile([P, D], f32, tag="m3")
                    nc.vector.tensor_mul(m3[:], hi, rr)
                    m4 = prod.tile([P, D], f32, tag="m4")
                    nc.vector.tensor_mul(m4[:], hr, ri)
                    m1 = prod.tile([P, D], f32, tag="m1")
                    nc.vector.tensor_mul(m1[:], hr, rr)
                    m2 = prod.tile([P, D], f32, tag="m2")
                    nc.vector.tensor_mul(m2[:], hi, ri)
                    ur = upool.tile([P, D], f32, tag="ur")
                    nc.vector.tensor_sub(ur[:], m1[:], m2[:])
                    ui = upool.tile([P, D], f32, tag="ui")
                    nc.vector.tensor_add(ui[:], m3[:], m4[:])

                    so1 = prod.tile([P, D], f32, tag="m1")
                    nc.vector.scalar_tensor_tensor(
                        out=so1[:], in0=ur[:], scalar=1.0, in1=tr,
                        op0=Alu.mult, op1=Alu.mult,
                        accum_out=ab1_s[:, t:t + 1],
                    )
                    so2 = prod.tile([P, D], f32, tag="m2")
                    nc.vector.scalar_tensor_tensor(
                        out=so2[:], in0=ui[:], scalar=-1.0, in1=ti,
                        op0=Alu.mult, op1=Alu.mult,
                        accum_out=ab2_s[:, t:t + 1],
                    )

                    for src, dst in (
                        (tr, bb1_s), (ti, bb2_s),
                        (ur[:], aa1_s), (ui[:], aa2_s),
                    ):
                        ao = scr.tile([P, D], f32, tag="ao")
                        nc.scalar.activation(
                            out=ao[:], in_=src, func=Act.Square,
                            accum_out=dst[:, t:t + 1],
                        )

            if dma_only:
                score = stats.tile([P, NT], f32, tag="score")
                nc.vector.memset(score[:], 0.0)
                nc.sync.dma_start(out=out_d, in_=score[:])
                return

            fin = {}
            def ftile(name):
                tl = stats.tile([P, NT], f32, tag=name)
                fin[name] = tl
                return tl

            ab = ftile("ab"); nc.vector.tensor_add(ab[:], ab1_s[:], ab2_s[:])
            aa = ftile("aa"); nc.vector.tensor_add(aa[:], aa1_s[:], aa2_s[:])
            bb = ftile("bb"); nc.vector.tensor_add(bb[:], bb1_s[:], bb2_s[:])
            pp = ftile("pp"); nc.vector.tensor_mul(pp[:], aa[:], bb[:])
            r = ftile("r0"); nc.scalar.activation(out=r[:], in_=pp[:], func=Act.Sqrt)
            for it in range(2):
                q = ftile(f"q{it}"); nc.vector.reciprocal(q[:], r[:])
                pq = ftile(f"pq{it}"); nc.vector.tensor_mul(pq[:], pp[:], q[:])
                s = ftile(f"s{it}"); nc.vector.tensor_add(s[:], r[:], pq[:])
                r = ftile(f"r{it + 1}"); nc.vector.tensor_scalar_mul(r[:], s[:], 0.5)
            inv = ftile("inv"); nc.vector.reciprocal(inv[:], r[:])
            score = ftile("score"); nc.vector.tensor_mul(score[:], ab[:], inv[:])
            nc.sync.dma_start(out=out_d, in_=score[:])
        return

    if cfg in ("dmaonly", "dmaonly3", "dmaonly4", "dmaonly12"):
        # Diagnostic: input streaming only — the pure DMA/HBM floor.
        flat_nb = {"dmaonly3": 3, "dmaonly4": 4}.get(cfg)
        half = cfg == "dmaonly12"
        with (
            tc.tile_pool(name="inp", bufs=2) as inp,
            tc.tile_pool(name="stats", bufs=1) as stats,
        ):
            for _rep in range(repeats):
                for t in range(NT):
                    for n in IN_NAMES:
                        nb = flat_nb or (3 if n.startswith("tail") else 2)
                        tl = inp.tile([P, D], f32, tag=n, bufs=nb)
                        if half:
                            h = D // 2
                            nc.sync.dma_start(out=tl[:, :h],
                                              in_=dv[n][t][:, :h])
                            nc.sync.dma_start(out=tl[:, h:],
                                              in_=dv[n][t][:, h:])
                        else:
                            nc.sync.dma_start(out=tl[:], in_=dv[n][t])
            score = stats.tile([P, NT], f32, tag="score")
            nc.vector.memset(score[:], 0.0)
            nc.sync.dma_start(out=out_d, in_=score[:])
        return

    if cfg == "computeonly":
        # Diagnostic: same compute chain, no input DMAs (static tiles).
        with (
            tc.tile_pool(name="inp", bufs=1) as inp,
            tc.tile_pool(name="prod", bufs=1) as prod,
            tc.tile_pool(name="upool", bufs=2) as upool,
            tc.tile_pool(name="scr", bufs=1) as scr,
            tc.tile_pool(name="stats", bufs=1) as stats,
        ):
            tiles = {}
            for n in IN_NAMES:
                tl = inp.tile([P, D], f32, tag=n)
                nc.vector.memset(tl[:], 0.25)
                tiles[n] = tl
            hr, hi = tiles["head_real"], tiles["head_imag"]
            rr, ri = tiles["rel_real"], tiles["rel_imag"]
            tr, ti = tiles["tail_real"], tiles["tail_imag"]
            ab1_s = stats.tile([P, NT], f32, tag="ab1_s")
            ab2_s = stats.tile([P, NT], f32, tag="ab2_s")
            aa1_s = stats.tile([P, NT], f32, tag="aa1_s")
            aa2_s = stats.tile([P, NT], f32, tag="aa2_s")
            bb1_s = stats.tile([P, NT], f32, tag="bb1_s")
            bb2_s = stats.tile([P, NT], f32, tag="bb2_s")
            for _rep in range(repeats):
                for t in range(NT):
                    m3 = prod.tile([P, D], f32, tag="m3")
                    nc.vector.tensor_mul(m3[:], hi[:], rr[:])
                    m4 = prod.tile([P, D], f32, tag="m4")
                    nc.vector.tensor_mul(m4[:], hr[:], ri[:])
                    m1 = prod.tile([P, D], f32, tag="m1")
                    nc.vector.tensor_mul(m1[:], hr[:], rr[:])
                    m2 = prod.tile([P, D], f32, tag="m2")
                    nc.vector.tensor_mul(m2[:], hi[:], ri[:])
                    ur = upool.tile([P, D], f32, tag="ur")
                    nc.vector.tensor_sub(ur[:], m1[:], m2[:])
                    ui = upool.tile([P, D], f32, tag="ui")
                    nc.vector.tensor_add(ui[:], m3[:], m4[:])
                    so1 = prod.tile([P, D], f32, tag="m1")
                    nc.vector.scalar_tensor_tensor(
                        out=so1[:], in0=ur[:], scalar=1.0, in1=tr[:],
                        op0=Alu.mult, op1=Alu.mult,
                        accum_out=ab1_s[:, t:t + 1],
                    )
                    so2 = prod.tile([P, D], f32, tag="m2")
                    nc.vector.scalar_tensor_tensor(
                        out=so2[:], in0=ui[:], scalar=-1.0, in1=ti[:],
                        op0=Alu.mult, op1=Alu.mult,
                        accum_out=ab2_s[:, t:t + 1],
                    )
                    for src, dst in (
                        (tr, bb1_s), (ti, bb2_s), (ur, aa1_s), (ui, aa2_s),
                    ):
                        ao = scr.tile([P, D], f32, tag="ao")
                        nc.scalar.activation(
                            out=ao[:], in_=src[:], func=Act.Square,
                            accum_out=dst[:, t:t + 1],
                        )
            score = stats.tile([P, NT], f32, tag="score")
            nc.vector.memset(score[:], 0.0)
            nc.sync.dma_start(out=out_d, in_=score[:])
        return

    with (
        tc.tile_pool(name="inp", bufs=2) as inp,
        tc.tile_pool(name="prod", bufs=1) as prod,
        tc.tile_pool(name="upool", bufs=2) as upool,
        tc.tile_pool(name="scr", bufs=1) as scr,
        tc.tile_pool(name="stats", bufs=1) as stats,
    ):
        ab1_s = stats.tile([P, NT], f32, tag="ab1_s")
        ab2_s = stats.tile([P, NT], f32, tag="ab2_s")
        aa1_s = stats.tile([P, NT], f32, tag="aa1_s")
        aa2_s = stats.tile([P, NT], f32, tag="aa2_s")
        bb1_s = stats.tile([P, NT], f32, tag="bb1_s")
        bb2_s = stats.tile([P, NT], f32, tag="bb2_s")

        for _rep in range(repeats):
          for t in range(NT):
            tiles = {}
            issue_order = IN_NAMES
            if cfg == "v10":
                issue_order = IN_NAMES[4:] + IN_NAMES[:4]
            for i, n in enumerate(issue_order):
                # tail tiles are the last-released each tile (read by the
                # STT dots at the end) — give them one extra buffer so
                # their next DMA isn't gated on the ring.
                if cfg in ("v5", "v9", "v10"):
                    nb = 3
                elif cfg == "v7":
                    nb = 4 if n.startswith("tail") else 2
                else:
                    nb = 3 if n.startswith("tail") else 2
                tl = inp.tile([P, D], f32, tag=n, bufs=nb)
                # v8/v9: alternate the two HWDGE rings (SP and ACT) so
                # descriptor generation / completion handling overlap.
                eng = nc.scalar if (cfg in ("v8", "v9") and i % 2) else nc.sync
                eng.dma_start(out=tl[:], in_=dv[n][t])
                tiles[n] = tl
            hr, hi = tiles["head_real"], tiles["head_imag"]
            rr, ri = tiles["rel_real"], tiles["rel_imag"]
            tr, ti = tiles["tail_real"], tiles["tail_imag"]

            # All products on DVE: GPSIMD's fp32 tensor_tensor measured
            # ~4.5x slower than DVE here and coupling it into the tile
            # pipeline made the kernel slower, not faster.
            m3 = prod.tile([P, D], f32, tag="m3")
            nc.vector.tensor_mul(m3[:], hi[:], rr[:])
            m4 = prod.tile([P, D], f32, tag="m4")
            nc.vector.tensor_mul(m4[:], hr[:], ri[:])
            m1 = prod.tile([P, D], f32, tag="m1")
            nc.vector.tensor_mul(m1[:], hr[:], rr[:])
            m2 = prod.tile([P, D], f32, tag="m2")
            nc.vector.tensor_mul(m2[:], hi[:], ri[:])
            ub = 1 if cfg in ("v5", "v9", "v10") else 2
            ur = upool.tile([P, D], f32, tag="ur", bufs=ub)
            nc.vector.tensor_sub(ur[:], m1[:], m2[:])
            ui = upool.tile([P, D], f32, tag="ui", bufs=ub)
            nc.vector.tensor_add(ui[:], m3[:], m4[:])

            # ab = sum(ur*tr) - sum(ui*ti): fused multiply+reduce via
            # scalar_tensor_tensor (out = (in0 op0 scalar) op1 in1,
            # accum_out = sum(out)). tensor_tensor_reduce (native TTR
            # opcode) crashes this terminal's NRT — do not use it.
            # Scratch outs alias the dead m1/m2 slots (same pool tag) —
            # WAR/WAW stay on-engine, zero extra SBUF.
            so1 = prod.tile([P, D], f32, tag="m1")
            nc.vector.scalar_tensor_tensor(
                out=so1[:], in0=ur[:], scalar=1.0, in1=tr[:],
                op0=Alu.mult, op1=Alu.mult, accum_out=ab1_s[:, t:t + 1],
            )
            so2 = prod.tile([P, D], f32, tag="m2")
            nc.vector.scalar_tensor_tensor(
                out=so2[:], in0=ui[:], scalar=-1.0, in1=ti[:],
                op0=Alu.mult, op1=Alu.mult, accum_out=ab2_s[:, t:t + 1],
            )

            # aa, bb: square+accumulate on ACT. bb first — tr/ti are
            # already resident before DVE finishes the products, so ACT
            # starts early and tr/ti stay hot for the STT dots.
            for src, dst in (
                (tr, bb1_s), (ti, bb2_s), (ur, aa1_s), (ui, aa2_s),
            ):
                ao = scr.tile([P, D], f32, tag="ao")
                nc.scalar.activation(
                    out=ao[:], in_=src[:], func=Act.Square,
                    accum_out=dst[:, t:t + 1],
                )

        # Final combine on [P, NT] (tiny).
        fin = {}
        def ftile(name):
            tl = stats.tile([P, NT], f32, tag=name)
            fin[name] = tl
            return tl

        ab = ftile("ab"); nc.vector.tensor_add(ab[:], ab1_s[:], ab2_s[:])
        aa = ftile("aa"); nc.vector.tensor_add(aa[:], aa1_s[:], aa2_s[:])
        bb = ftile("bb"); nc.vector.tensor_add(bb[:], bb1_s[:], bb2_s[:])
        pp = ftile("pp"); nc.vector.tensor_mul(pp[:], aa[:], bb[:])
        # sqrt on ACT is low precision (up to ~65536 ULP budget); refine
        # with two Newton iterations  r <- 0.5*(r + p/r)  using the
        # bit-exact DVE reciprocal.
        r = ftile("r0"); nc.scalar.activation(out=r[:], in_=pp[:], func=Act.Sqrt)
        for it in range(2):
            q = ftile(f"q{it}"); nc.vector.reciprocal(q[:], r[:])
            pq = ftile(f"pq{it}"); nc.vector.tensor_mul(pq[:], pp[:], q[:])
            s = ftile(f"s{it}"); nc.vector.tensor_add(s[:], r[:], pq[:])
            r = ftile(f"r{it + 1}"); nc.vector.tensor_scalar_mul(r[:], s[:], 0.5)
        inv = ftile("inv"); nc.vector.reciprocal(inv[:], r[:])
        score = ftile("score"); nc.vector.tensor_mul(score[:], ab[:], inv[:])
        nc.sync.dma_start(out=out_d, in_=score[:])


def _build(repeats=1, cfg="v4"):
    key = ("nc", repeats, cfg)
    if key in _CACHE:
        return _CACHE[key]
    import concourse.tile as tile
    from concourse import bacc, mybir

    # NOTE: num_devices is deliberately NOT set — it enables collective
    # global-comm setup that breaks plain SPMD input binding under the
    # axon/PJRT path (outputs come back as garbage).
    nc = bacc.Bacc(
        "TRN2",
        target_bir_lowering=False,
        debug=False,
    )
    if cfg in PACKED_CFGS:
        ins = {
            "packed": nc.dram_tensor(
                "packed", [NTEN * BC, D], mybir.dt.float32,
                kind="ExternalInput",
            ).ap()
        }
    else:
        ins = {
            n: nc.dram_tensor(n, [BC, D], mybir.dt.float32, kind="ExternalInput").ap()
            for n in IN_NAMES
        }
    out = nc.dram_tensor("out", [BC], mybir.dt.float32, kind="ExternalOutput").ap()
    with tile.TileContext(nc) as tc:
        _emit(tc, ins, out, mybir, repeats=repeats, cfg=cfg)
    nc.compile()
    _CACHE[key] = nc
    return nc


def run(inputs, trace=False, cfg=DEFAULT_CFG, **kwargs):
    """Run on 8 cores; returns (full_output, BassKernelResults)."""
    from concourse.bass_utils import run_bass_kernel_spmd

    nc = _build(cfg=cfg)
    core_ids = list(range(NCORES))
    in_maps = [host_prep(inputs, c, cfg) for c in range(NCORES)]
    # The terminal occasionally reports the accelerator unrecoverable
    # (e.g. poisoned by an earlier crashed run); a fresh attempt after a
    # short wait triggers recovery.
    last_exc = None
    for attempt in range(4):
        try:
            res = run_bass_kernel_spmd(nc, in_maps, core_ids, trace=trace, **kwargs)
            break
        except Exception as e:  # noqa: BLE001
            last_exc = e
            if attempt == 3:
                raise
            import time as _time
            _time.sleep(15 * (attempt + 1))
    out = np.concatenate([res.results[c]["out"] for c in range(NCORES)])
    return out.astype(np.float32), res


def kernel(**inputs):
    out, _ = run(inputs)
    return out

